# revision 24
# baseline (speedup 1.0000x reference)
"""3-layer GAT (PPI-style) forward on 8 Trainium2 NeuronCores.

Strategy (SPMD, one NEFF on 8 cores):
  - Host: add self-loops, degree-balanced node permutation into 8 cores x
    2500 nodes (tiles of 128 dst nodes), edges sorted by dst and padded to a
    uniform chunk count; int16 gather-index arrays precomputed; transposed
    one-hot matrices (dst-slot -> edge) precomputed in fp8.
  - Attention dot products a_s.h / a_d.h are folded into the dense matmul as
    extra weight columns (wES = W_h @ a_s[h]), so es/ed come out of the PE.
  - Per layer: dense phase [h|es|ed|lin] = x @ waug with SBUF-resident tiled
    lhsT (xT layout maintained across layers via PE transposes); payload rows
    [h0|1|h1|1|...|es] (bf16 + f32 es) staged to DRAM; chunked AllGather
    (4 pieces, overlaps dense); aggregation: dma_gather payload[src] per edge,
    per-edge dst logit via PE matmul with fp8 one-hot^T, w = exp(lrelu(es+ed))
    on the scalar engine, scaled one-hot lhsT built on DVE from a bf16 iota
    (4x mode), PE matmul accumulates per-head numerators + denominator
    (interleaved ones column), normalize via scalar engine, add skip + bias,
    ELU -> next layer input transposed on PE back to xT layout.
"""

import math
import numpy as np

N_CORES = 8
USE_PRELU = False
GROUP = 6   # gather chunks per dma_gather group
NAG = 1     # AllGather pieces (tile-aligned)


# --------------------------------------------------------------------------
# host-side prep (pure data layout / graph partitioning, no model math)
# --------------------------------------------------------------------------

def _balance_permutation(dst, n, n_cores, tiles_per_core, rows_last):
    """Greedy balance: nodes -> 128-row dst tiles with ~equal edge counts.
    Within each tile, nodes are split over two 64-slot halves with ~equal
    edge counts (the aggregation works on 64-row one-hot windows)."""
    import heapq

    deg = np.bincount(dst, minlength=n).astype(np.int64)
    order = np.argsort(-deg, kind="stable")
    n_tiles = n_cores * tiles_per_core
    caps = np.full(n_tiles, 128, np.int64)
    caps[tiles_per_core - 1 :: tiles_per_core] = rows_last
    heap = [(0, int(b)) for b in range(n_tiles)]
    heapq.heapify(heap)
    members = [[] for _ in range(n_tiles)]
    loads = np.zeros(n_tiles, np.int64)
    for node in order:
        while True:
            load, b = heapq.heappop(heap)
            if len(members[b]) < caps[b]:
                break
        members[b].append(node)
        loads[b] += deg[node]
        if len(members[b]) < caps[b]:
            heapq.heappush(heap, (int(loads[b]), b))
    perm_o2n = np.empty(n, np.int64)
    per_core = tiles_per_core * 128 - (128 - rows_last)
    for b in range(n_tiles):
        core, t = divmod(b, tiles_per_core)
        base = core * per_core + t * 128
        ids = np.asarray(members[b], np.int64)
        cap = int(caps[b])
        cap0 = min(64, cap)
        # split members over the two 64-slot halves, balancing edges
        h_load = [0, 0]
        h_free = [cap0, cap - cap0]
        half_of = {}
        for node in sorted(ids, key=lambda v: -deg[v]):
            hsel = 0 if (h_free[0] > 0 and (h_load[0] <= h_load[1] or h_free[1] == 0)) else 1
            half_of[node] = hsel
            h_load[hsel] += int(deg[node])
            h_free[hsel] -= 1
        s0, s1 = 0, cap0
        for node in ids:
            if half_of[node] == 0:
                perm_o2n[node] = base + s0
                s0 += 1
            else:
                perm_o2n[node] = base + s1
                s1 += 1
    return perm_o2n


def _wrap16_rep(a):
    """[L] int -> [128, L/16] int16 (16-wrap, replicated 8x down partitions)."""
    w = a.reshape(-1, 16).T.astype(np.int16)
    return np.ascontiguousarray(np.tile(w, (8, 1)))


def _ag_chunks(T, rows_of):
    """Tile-aligned AllGather pieces: list of (tile_lo, tile_hi, rows)."""
    tpc = math.ceil(T / NAG)
    chunks = []
    for a in range(NAG):
        lo, hi = a * tpc, min((a + 1) * tpc, T)
        if lo >= hi:
            break
        chunks.append((lo, hi, sum(rows_of(t) for t in range(lo, hi))))
    return chunks


def _host_prep(inputs, n_cores=N_CORES):
    import ml_dtypes

    bf16 = ml_dtypes.bfloat16
    f8 = ml_dtypes.float8_e4m3fn
    x = np.asarray(inputs["x"], np.float32)
    ei = np.asarray(inputs["edge_index"])
    n, f_in = x.shape
    loop = np.arange(n, dtype=ei.dtype)
    src = np.concatenate([ei[0], loop]).astype(np.int64)
    dst = np.concatenate([ei[1], loop]).astype(np.int64)

    per_core = n // n_cores
    tiles_per_core = math.ceil(per_core / 128)
    rows_last = per_core - (tiles_per_core - 1) * 128
    T = tiles_per_core

    def rows_of(t):
        return 128 if t < T - 1 else rows_last

    perm = _balance_permutation(dst, n, n_cores, T, rows_last)
    src_n = perm[src]
    dst_n = perm[dst]

    # pfull row remap for chunked AllGather: row' for node at (core, r)
    chunks = _ag_chunks(T, rows_of)
    # per-core row -> (chunk idx, offset within chunk rows)
    r2chunk = np.empty(per_core, np.int64)
    r2off = np.empty(per_core, np.int64)
    ch_base = []  # global pfull base row of chunk a
    base = 0
    row = 0
    for a, (lo, hi, rows_a) in enumerate(chunks):
        r2chunk[row : row + rows_a] = a
        r2off[row : row + rows_a] = np.arange(rows_a)
        ch_base.append(base)
        base += n_cores * rows_a
        row += rows_a
    assert row == per_core and base == n

    def pfull_row(node_new):
        c, r = node_new // per_core, node_new % per_core
        a = r2chunk[r]
        rows_a = np.array([chunks[int(ai)][2] for ai in a])
        return np.array(ch_base)[a] + c * rows_a + r2off[r]

    core_of = dst_n // per_core
    per_ctb_src = {}
    per_ctb_loc = {}
    cnt_b = np.zeros((n_cores, T, 2), np.int64)
    for c in range(n_cores):
        sel = core_of == c
        s, d = src_n[sel], dst_n[sel]
        locv = d - c * per_core
        o = np.argsort(locv, kind="stable")
        s, locv = s[o], locv[o]
        tile_of = locv // 128
        for t in range(T):
            m = tile_of == t
            st_, lt_ = s[m], locv[m] - t * 128
            for b in range(2):
                mb = (lt_ // 64) == b
                per_ctb_src[c, t, b] = st_[mb]
                per_ctb_loc[c, t, b] = lt_[mb]
                cnt_b[c, t, b] = mb.sum()

    # per-tile per-block chunk counts (uniform over cores for SPMD)
    nch_b = np.maximum(np.ceil(cnt_b.max(axis=0) / 128).astype(np.int64), 1)
    nchunk_t = nch_b.sum(axis=1)          # [T]
    nchmax = int(nchunk_t.max())
    group = GROUP

    src16 = np.zeros((n_cores, T, 128, nchmax * 8), np.int16)
    dloc = np.full((n_cores, T, 128, nchmax), -1.0, np.float32)
    otr = np.zeros((n_cores, T, 128, nchmax * 128), np.float32)
    for c in range(n_cores):
        for t in range(T):
            nct = int(nchunk_t[t])
            cap = nct * 128
            ps = np.zeros(cap, np.int64)
            pl = np.full(cap, -1, np.int64)    # absolute loc (for OT)
            plw = np.full(cap, -1, np.int64)   # block-relative loc (for ohw)
            off = 0
            for b in range(2):
                e = int(cnt_b[c, t, b])
                if e:
                    ps[off : off + e] = pfull_row(per_ctb_src[c, t, b])
                    pl[off : off + e] = per_ctb_loc[c, t, b]
                    plw[off : off + e] = per_ctb_loc[c, t, b] - 64 * b
                off += int(nch_b[t, b]) * 128
            src16[c, t, :, : nct * 8] = _wrap16_rep(ps)
            dloc[c, t, :, :nct] = plw.reshape(nct, 128).T.astype(np.float32)
            pl2 = pl.reshape(nct, 128)
            ot = np.zeros((128, nct, 128), np.float32)
            dmask = pl2 >= 0
            dd = np.where(dmask, pl2, 0)
            chn, en = np.nonzero(dmask)
            ot[dd[chn, en], chn, en] = 1.0
            otr[c, t, :, : nct * 128] = ot.reshape(128, nct * 128)

    h1, c1 = np.asarray(inputs["a1s"]).shape
    h3, c3 = np.asarray(inputs["a3s"]).shape
    d1 = h1 * c1

    g = lambda k: np.asarray(inputs[k], np.float32)

    def build_waug(W, Wl, a_s, a_d, heads, ch):
        # [W | wES(H) | wED(H) | Wl]
        wes = np.stack([W[:, h * ch : (h + 1) * ch] @ a_s[h] for h in range(heads)], 1)
        wed = np.stack([W[:, h * ch : (h + 1) * ch] @ a_d[h] for h in range(heads)], 1)
        return np.concatenate([W, wes, wed, Wl], 1).astype(bf16)

    waug1 = build_waug(g("W1"), g("Wl1"), g("a1s"), g("a1d"), h1, c1)
    waug2 = build_waug(g("W2"), g("Wl2"), g("a2s"), g("a2d"), h1, c1)
    waug3 = build_waug(g("W3"), g("Wl3"), g("a3s"), g("a3d"), h3, c3)
    # pad L1 rows 50 -> 64
    kpad1 = 64
    waug1 = np.concatenate(
        [waug1, np.zeros((kpad1 - f_in, waug1.shape[1]), bf16)], 0
    )

    rep = lambda v: np.ascontiguousarray(np.broadcast_to(v[None, :], (128, v.shape[0])))
    bsum1 = rep(g("b1") + g("bl1")).astype(np.float32)
    bsum2 = rep(g("b2") + g("bl2")).astype(np.float32)
    bsum3 = rep(g("b3") + g("bl3")).astype(np.float32)

    # permuted node features, transposed+tiled, padded, bf16, per core
    rows_pad = T * 128
    x_perm = np.zeros((n, f_in), np.float32)
    x_perm[perm] = x
    xT1 = []
    for c in range(n_cores):
        blk = np.zeros((rows_pad, f_in), np.float32)
        blk[:per_core] = x_perm[c * per_core : (c + 1) * per_core]
        xt = np.zeros((kpad1, rows_pad), np.float32)
        xt[:f_in] = blk.T
        xT1.append(np.ascontiguousarray(xt).astype(bf16))

    base_map = dict(
        waug1=waug1, waug2=waug2, waug3=waug3,
        bsum1=bsum1, bsum2=bsum2, bsum3=bsum3,
    )
    in_maps = []
    for c in range(n_cores):
        m = dict(base_map)
        m["xT1"] = xT1[c]
        m["src16"] = src16[c]
        m["dloc"] = dloc[c]
        m["otr"] = otr[c].astype(f8)
        in_maps.append(m)

    cfg = dict(
        n=n, f_in=f_in, kpad1=kpad1, n_cores=n_cores, per_core=per_core,
        tiles_per_core=T, rows_last=rows_last, rows_pad=rows_pad,
        nchmax=nchmax, nch_b=nch_b.tolist(), nchunk_t=nchunk_t.tolist(),
        group=group, chunks=chunks, ch_base=ch_base,
        h1=h1, c1=c1, d1=d1, h3=h3, c3=c3,
    )
    return in_maps, cfg, perm


# --------------------------------------------------------------------------
# bass program
# --------------------------------------------------------------------------

def _layer_dims(cfg):
    """Static per-layer dims.

    Payload row: [h0|1|h1|1|...] (H*(C+1) fp8 bytes) then es (H f32), padded
    so the byte width is a multiple of 256 (dma_gather elem constraint)."""
    out = []
    for li in (1, 2, 3):
        if li < 3:
            h, c = cfg["h1"], cfg["c1"]
            din = cfg["kpad1"] if li == 1 else cfg["d1"]
            nlin = cfg["d1"]
        else:
            h, c = cfg["h3"], cfg["c3"]
            din = cfg["d1"]
            nlin = cfg["c3"]
        st = c + 1
        unit = 2                          # payload h dtype bytes (bf16)
        hst = h * st                      # payload [h|1] region, in units
        es_b = hst * unit                 # byte offset of es (bf16 now)
        pw = math.ceil((es_b + h * 2) / 256) * 256   # bytes
        hc = h * c
        naug = hc + 2 * h + nlin          # [W | wES | wED | Wl]
        kch = math.ceil(din / 128)
        out.append(dict(li=li, din=din, kch=kch, naug=naug, nlin=nlin,
                        h=h, c=c, st=st, hst=hst, es=es_b // 4, pw=pw,
                        hc=hc, unit=unit))
    return out


def _build(cfg):
    import concourse.bass as bass
    import concourse.bacc as bacc
    import concourse.mybir as mybir
    import concourse.tile as tile
    from contextlib import ExitStack

    f32 = mybir.dt.float32
    bf = mybir.dt.bfloat16
    f8 = mybir.dt.float8e4
    i16 = mybir.dt.int16
    i32 = mybir.dt.int32
    u16 = mybir.dt.uint16
    u8 = mybir.dt.uint8
    EXP = mybir.ActivationFunctionType.Exp
    PRELU = mybir.ActivationFunctionType.Prelu
    LRELU = mybir.ActivationFunctionType.Lrelu
    COPY = mybir.ActivationFunctionType.Copy
    ALU = mybir.AluOpType

    n_cores = cfg["n_cores"]
    n = cfg["n"]
    T = cfg["tiles_per_core"]
    rows_last = cfg["rows_last"]
    per_core = cfg["per_core"]
    NCHMAX = cfg["nchmax"]
    NCH_B = cfg["nch_b"]          # [T][2]
    NCHUNK_T = cfg["nchunk_t"]    # [T]
    GRP = cfg["group"]
    D1 = cfg["d1"]
    chunks = cfg["chunks"]
    layers = _layer_dims(cfg)
    PWMAX = max(L["pw"] for L in layers)

    nc = bacc.Bacc(None, target_bir_lowering=False, dynamic_dma_scratch_size=49152)

    # ---- parameters -----------------------------------------------------
    xT1 = nc.declare_dram_parameter("xT1", [cfg["kpad1"], cfg["rows_pad"]], bf, isOutput=False)
    waug_p = {
        1: nc.declare_dram_parameter("waug1", [cfg["kpad1"], layers[0]["naug"]], bf, isOutput=False),
        2: nc.declare_dram_parameter("waug2", [D1, layers[1]["naug"]], bf, isOutput=False),
        3: nc.declare_dram_parameter("waug3", [D1, layers[2]["naug"]], bf, isOutput=False),
    }
    bsum_p = {}
    for li, L in ((1, layers[0]), (2, layers[1]), (3, layers[2])):
        bsum_p[li] = nc.declare_dram_parameter(f"bsum{li}", [128, L["nlin"]], f32, isOutput=False)
    src16_p = nc.declare_dram_parameter("src16", [T, 128, NCHMAX * 8], i16, isOutput=False)
    dloc_p = nc.declare_dram_parameter("dloc", [T, 128, NCHMAX], f32, isOutput=False)
    otr_p = nc.declare_dram_parameter("otr", [T, 128, NCHMAX * 128], f8, isOutput=False)
    out_p = nc.declare_dram_parameter("out", [per_core, cfg["c3"]], f32, isOutput=True)

    with tile.TileContext(nc, num_cores=n_cores) as tc, ExitStack() as ctx:
        # ---- dram scratch ----------------------------------------------
        dram = ctx.enter_context(tc.tile_pool(name="dram", bufs=1, space="DRAM"))
        pshard = {}
        for L in layers:
            li = L["li"]
            for a, (lo, hi, rows_a) in enumerate(chunks):
                pshard[li, a] = dram.tile([rows_a, L["pw"]], u8,
                                          tag=f"psh{li}_{a}", name=f"psh{li}_{a}")
        pfull = {L["li"]: dram.tile([n, L["pw"]], u8, tag=f"pfull{L['li']}",
                                    name=f"pfull{L['li']}", addr_space="Shared")
                 for L in layers}
        xtd = {li: dram.tile([T, 128, D1], bf, tag=f"xtd{li}", name=f"xtd{li}")
               for li in (1, 2)}
        linb = {}
        for L in layers:
            li = L["li"]
            dt_lin = f32 if li == 3 else bf
            linb[li] = dram.tile([T, 128, L["nlin"]], dt_lin, tag=f"lin{li}", name=f"lin{li}")

        # ---- pools ------------------------------------------------------
        consts = ctx.enter_context(tc.tile_pool(name="consts", bufs=1))
        waugp = ctx.enter_context(tc.tile_pool(name="waugp", bufs=1))
        bsump = ctx.enter_context(tc.tile_pool(name="bsump", bufs=2))
        edtp = ctx.enter_context(tc.tile_pool(name="edtp", bufs=2))
        xtp = ctx.enter_context(tc.tile_pool(name="xtp", bufs=2))
        ptp = ctx.enter_context(tc.tile_pool(name="ptp", bufs=3))
        ltp = ctx.enter_context(tc.tile_pool(name="ltp", bufs=2))
        idxp = ctx.enter_context(tc.tile_pool(name="idxp", bufs=3))
        otp = ctx.enter_context(tc.tile_pool(name="otp", bufs=3))
        gp = ctx.enter_context(tc.tile_pool(name="gp", bufs=4))
        lgp = ctx.enter_context(tc.tile_pool(name="lgp", bufs=3))
        ohwp = ctx.enter_context(tc.tile_pool(name="ohwp", bufs=12))
        scp = ctx.enter_context(tc.tile_pool(name="scp", bufs=6))
        epip = ctx.enter_context(tc.tile_pool(name="epip", bufs=2))
        recp = ctx.enter_context(tc.tile_pool(name="recp", bufs=8))
        xsp = ctx.enter_context(tc.tile_pool(name="xsp", bufs=2))
        psum_d = ctx.enter_context(tc.tile_pool(name="psum_d", bufs=2, space="PSUM"))
        psum_a = ctx.enter_context(tc.tile_pool(name="psum_a", bufs=1, space="PSUM"))
        psum_t = ctx.enter_context(tc.tile_pool(name="psum_t", bufs=1, space="PSUM"))

        # ---- constants ---------------------------------------------------
        iota_i = consts.tile([128, 128], i32, tag="iota_i")
        nc.gpsimd.iota(iota_i[:, :], pattern=[[1, 128]], base=0, channel_multiplier=0)
        iota_bf = consts.tile([128, 128], bf, tag="iota_bf")
        nc.vector.tensor_copy(iota_bf[:, :], iota_i[:, :])
        iotac_i = consts.tile([128, 1], i32, tag="iotac_i")
        nc.gpsimd.iota(iotac_i[:, :], pattern=[[0, 1]], base=0, channel_multiplier=1)
        iotac_f = consts.tile([128, 1], f32, tag="iotac_f")
        nc.vector.tensor_copy(iotac_f[:, :], iotac_i[:, :])
        ident_bf = consts.tile([128, 128], bf, tag="ident_bf")
        nc.vector.tensor_scalar(
            out=ident_bf[:, :], in0=iota_bf[:, :], scalar1=iotac_f[:, 0:1],
            scalar2=None, op0=ALU.is_equal,
        )
        # xT1 resident (tiny)
        xt1_sb = consts.tile([cfg["kpad1"], cfg["rows_pad"]], bf, tag="xt1")
        nc.sync.dma_start(out=xt1_sb[:, :], in_=xT1[:, :])

        def rows_of(t):
            return 128 if t < T - 1 else rows_last

        # ------------------------------------------------------------------
        for L in layers:
            li, DIN, KCH, NAUG = L["li"], L["din"], L["kch"], L["naug"]
            H, C, ST, HST = L["h"], L["c"], L["st"], L["hst"]
            PW, ES = L["pw"], L["es"]
            NLIN, HC = L["nlin"], L["hc"]
            pdt = f8 if L["unit"] == 1 else bf

            # layer constants
            wt = [waugp.tile([128, NAUG], bf, tag=f"waug_kc{k}", name=f"waug_kc{k}")
                  for k in range(KCH)]
            for k in range(KCH):
                kk = min(128, DIN - k * 128)
                nc.sync.dma_start(out=wt[k][:kk, :], in_=waug_p[li][k * 128 : k * 128 + kk, :])
            bsum = bsump.tile([128, NLIN], f32, tag="bsum")
            nc.sync.dma_start(out=bsum[:, :], in_=bsum_p[li][:, :])
            edt = edtp.tile([128, T * H], bf, tag="edt")

            # ---------------- dense phase --------------------------------
            # pass-A blocks: multiples of C so payload copies stay affine
            if li < 3:
                ablocks = [(0, 512, 2), (512, 1024, 2)]       # (col0, col1, heads)
            else:
                ablocks = [(0, 484, 4), (484, 726, 2)]
            nblocks = [(b0, min(b0 + 512, NLIN)) for b0 in range(0, NLIN, 512)]
            for t in range(T):
                r = rows_of(t)
                if li == 1:
                    lhs_of = lambda k, t=t: xt1_sb[:, t * 128 : (t + 1) * 128]
                else:
                    xtile = xtp.tile([128, KCH * 128], bf, tag="xtile", name="xtile")
                    nc.sync.dma_start(out=xtile[:, :], in_=xtd[li - 1][t])
                    lhs_of = lambda k, xtile=xtile: xtile[:, k * 128 : (k + 1) * 128]

                pt = ptp.tile([128, PW], u8, tag="pt")
                ptb = pt.bitcast(pdt)
                ptv = ptb[:, 0:HST].rearrange("p (h st) -> p h st", st=ST)
                ptf = pt.bitcast(f32)
                # pass A: W blocks -> payload h slots
                h0 = 0
                for (b0, b1, nh) in ablocks:
                    pd = psum_d.tile([128, 512], f32, tag="pd", name="pdA")
                    for k in range(KCH):
                        kk = min(128, DIN - k * 128)
                        nc.tensor.matmul(
                            pd[:, 0 : b1 - b0],
                            lhs_of(k),
                            wt[k][:kk, b0:b1],
                            start=(k == 0), stop=(k == KCH - 1),
                        )
                    nc.scalar.activation(
                        ptv[:, h0 : h0 + nh, 0:C],
                        pd[:, 0 : b1 - b0].rearrange("p (h c) -> p h c", c=C),
                        COPY,
                    )
                    h0 += nh
                # es/ed block
                pde = psum_d.tile([128, 512], f32, tag="pd", name="pde")
                for k in range(KCH):
                    kk = min(128, DIN - k * 128)
                    nc.tensor.matmul(
                        pde[:, 0 : 2 * H],
                        lhs_of(k),
                        wt[k][:kk, HC : HC + 2 * H],
                        start=(k == 0), stop=(k == KCH - 1),
                    )
                nc.vector.memset(ptv[:, :, C : C + 1], 1.0)
                nc.vector.tensor_copy(ptb[:, HST : HST + H], pde[:, 0:H])
                if 2 * (HST + H) < PW:
                    nc.vector.memset(pt[:, 2 * (HST + H) : PW], 0.0)
                nc.vector.tensor_copy(edt[:, t * H : (t + 1) * H], pde[:, H : 2 * H])
                a = min(t // math.ceil(T / NAG), len(chunks) - 1)
                lo, hi, rows_a = chunks[a]
                loc_r = sum(rows_of(tt) for tt in range(lo, t))
                nc.sync.dma_start(out=pshard[li, a][loc_r : loc_r + r, :], in_=pt[:r, :])

                # pass B: Wl -> lin staging
                lt = ltp.tile([128, NLIN], f32 if li == 3 else bf, tag="lt")
                for (b0, b1) in nblocks:
                    pdB = psum_d.tile([128, 512], f32, tag="pd", name="pdB")
                    for k in range(KCH):
                        kk = min(128, DIN - k * 128)
                        nc.tensor.matmul(
                            pdB[:, 0 : b1 - b0],
                            lhs_of(k),
                            wt[k][:kk, HC + 2 * H + b0 : HC + 2 * H + b1],
                            start=(k == 0), stop=(k == KCH - 1),
                        )
                    nc.vector.tensor_tensor(out=lt[:, b0:b1], in0=pdB[:, 0 : b1 - b0],
                                            in1=bsum[:, b0:b1], op=ALU.add)
                nc.sync.dma_start(out=linb[li][t, :, :], in_=lt[:, :])

                # fire AllGather piece when its tiles are done
                if t + 1 in [hi for (_, hi, _) in chunks]:
                    a2 = [hi for (_, hi, _) in chunks].index(t + 1)
                    lo2, hi2, rows_a2 = chunks[a2]
                    base2 = n_cores * sum(cc[2] for cc in chunks[:a2])
                    nc.gpsimd.collective_compute(
                        "AllGather",
                        ALU.bypass,
                        replica_groups=[list(range(n_cores))],
                        ins=[pshard[li, a2][:, :]],
                        outs=[pfull[li][base2 : base2 + n_cores * rows_a2, :]],
                    )

            pfull_t = pfull[li]

            # ---------------- aggregation phase --------------------------
            for t in range(T):
                r = rows_of(t)
                NCT = NCHUNK_T[t]
                NB0 = NCH_B[t][0]
                bstart = (0, NB0)
                blast = (NB0 - 1, NCT - 1)
                s16 = idxp.tile([128, NCHMAX * 8], i16, tag="s16")
                dlc = idxp.tile([128, NCHMAX], f32, tag="dlc")
                ots = otp.tile([128, NCHMAX * 128], f8, tag="ots")
                nc.sync.dma_start(out=s16[:, : NCT * 8], in_=src16_p[t][:, : NCT * 8])
                nc.sync.dma_start(out=dlc[:, :NCT], in_=dloc_p[t][:, :NCT])
                nc.sync.dma_start(out=ots[:, : NCT * 128], in_=otr_p[t][:, : NCT * 128])

                PSW = 512 if li < 3 else 256
                psall = psum_a.tile([128, 2048], f32, tag="psall", name="psall")
                edts = edt[:, t * H : (t + 1) * H]

                for g in range(math.ceil(NCT / GRP)):
                    GG = min(GRP, NCT - g * GRP)
                    G = gp.tile([128, GRP, PW], u8, tag="G")
                    nc.gpsimd.dma_gather(
                        out_ap=G[:, :GG, :],
                        in_ap=pfull_t[:, :],
                        idxs_ap=s16[:, g * GRP * 8 : (g * GRP + GG) * 8],
                        num_idxs=GG * 128,
                        num_idxs_reg=GG * 128,
                        elem_size=PW,
                    )
                    Gb = G.bitcast(pdt)
                    # per-edge logits: t = ed[dst] (one-hot^T matmul)
                    #                    + es[src] (identity matmul on payload)
                    ped = psum_t.tile([128, GRP * H], f32, tag="ped", name="ped")
                    for cch in range(GG):
                        j = g * GRP + cch
                        nc.tensor.matmul(
                            ped[:, cch * H : (cch + 1) * H],
                            ots[:, j * 128 : (j + 1) * 128],
                            edts,
                            start=True, stop=False,
                        )
                        nc.tensor.matmul(
                            ped[:, cch * H : (cch + 1) * H],
                            ident_bf[:, :],
                            Gb[:, cch, HST : HST + H],
                            start=False, stop=True,
                        )
                    pedv = ped.rearrange("p (g h) -> p g h", h=H)
                    we = lgp.tile([128, GRP, H], f32, tag="we")
                    halves = [(0, min(3, GG))] + ([(3, GG)] if GG > 3 else [])
                    for (c0, c1) in halves:
                        t2 = lgp.tile([128, GRP, H], f32, tag="t2")
                        nc.scalar.activation(t2[:, c0:c1, :], pedv[:, c0:c1, :], COPY,
                                             scale=0.2)
                        tr = lgp.tile([128, GRP, H], f32, tag="tr")
                        nc.vector.tensor_tensor(out=tr[:, c0:c1, :], in0=pedv[:, c0:c1, :],
                                                in1=t2[:, c0:c1, :], op=ALU.max)
                        nc.scalar.activation(we[:, c0:c1, :], tr[:, c0:c1, :], EXP)
                    HSPLIT = 2 if li < 3 else 3
                    for cch in range(GG):
                        j = g * GRP + cch
                        blk = 0 if j < NB0 else 1
                        pbase = 64 * blk
                        # heads [0:HSPLIT): scaled one-hot built on DVE
                        for h in range(HSPLIT):
                            ohw = ohwp.tile([128, 64], bf, tag="ohw")
                            nc.vector.tensor_scalar(
                                out=ohw[:, :], in0=iota_bf[:, 0:64],
                                scalar1=dlc[:, j : j + 1],
                                scalar2=we[:, cch, h : h + 1],
                                op0=ALU.is_equal, op1=ALU.mult,
                            )
                            nc.tensor.matmul(
                                psall[pbase : pbase + 64, h * PSW : h * PSW + ST],
                                ohw[:, :],
                                Gb[:, cch, h * ST : (h + 1) * ST],
                                start=(j == bstart[blk] and (li < 3 or h % 2 == 0)),
                                stop=(j == blast[blk] and (li < 3 or h % 2 == 1)),
                            )
                        # heads [HSPLIT:H): scalar engine scales the payload,
                        # shared unscaled one-hot
                        oun = ohwp.tile([128, 64], bf, tag="oun")
                        nc.vector.tensor_scalar(
                            out=oun[:, :], in0=iota_bf[:, 0:64],
                            scalar1=dlc[:, j : j + 1],
                            scalar2=None, op0=ALU.is_equal,
                        )
                        for h in range(HSPLIT, H):
                            scG = scp.tile([128, ST], bf, tag="scG")
                            nc.scalar.activation(
                                scG[:, :], Gb[:, cch, h * ST : (h + 1) * ST], COPY,
                                scale=we[:, cch, h : h + 1],
                            )
                            nc.tensor.matmul(
                                psall[pbase : pbase + 64, h * PSW : h * PSW + ST],
                                oun[:, :],
                                scG[:, :],
                                start=(j == bstart[blk] and (li < 3 or h % 2 == 0)),
                                stop=(j == blast[blk] and (li < 3 or h % 2 == 1)),
                            )

                # epilogue: one early PSUM->SBUF copy frees psall for the
                # next tile; reciprocals batched; normalize from the copy
                lt2 = ltp.tile([128, NLIN], f32 if li == 3 else bf, tag="lt2")
                nc.sync.dma_start(out=lt2[:, :], in_=linb[li][t])
                xps = epip.tile([128, H * ST], bf, tag="xps")
                xpsv = xps.rearrange("p (h st) -> p h st", st=ST)
                psv = psall[:, 0 : H * PSW].rearrange("p (h w) -> p h w", w=PSW)
                nc.scalar.activation(xpsv[:, :, :], psv[:, :, 0:ST], COPY)
                rec = recp.tile([128, H], f32, tag="rec")
                nc.vector.reciprocal(rec[:, :], xpsv[:, :, C])
                xt = epip.tile([128, HC], bf, tag="xt")
                for h in range(H):
                    nc.scalar.activation(
                        xt[:, h * C : (h + 1) * C], xps[:, h * ST : h * ST + C], COPY,
                        scale=rec[:, h : h + 1],
                    )
                if li < 3:
                    s = epip.tile([128, HC], bf, tag="s")
                    u = epip.tile([128, HC], bf, tag="u")
                    e = epip.tile([128, HC], bf, tag="e")
                    v = epip.tile([128, HC], bf, tag="v")
                    xn = epip.tile([128, HC], bf, tag="xn")
                    nc.vector.tensor_tensor(out=s[:, :], in0=xt[:, :], in1=lt2[:, :], op=ALU.add)
                    nc.vector.tensor_scalar(out=u[:, :], in0=s[:, :], scalar1=0.0,
                                            scalar2=None, op0=ALU.min)
                    nc.scalar.activation(e[:, :], u[:, :], EXP)
                    nc.vector.tensor_scalar(out=v[:, :], in0=s[:, :], scalar1=0.0,
                                            scalar2=-1.0, op0=ALU.max, op1=ALU.add)
                    nc.vector.tensor_tensor(out=xn[:, :], in0=v[:, :], in1=e[:, :], op=ALU.add)
                    # transpose back to xT layout for next layer's lhsT
                    xstage = xsp.tile([128, HC], bf, tag="xstage")
                    tp8 = psum_t.tile([128, 1024], bf, tag="tp8", name="tp8")
                    for k in range(math.ceil(HC / 128)):
                        nc.tensor.transpose(tp8[:, k * 128 : (k + 1) * 128],
                                            xn[:, k * 128 : (k + 1) * 128],
                                            ident_bf[:, :])
                    nc.vector.tensor_copy(
                        xstage.bitcast(i32)[:, :],
                        tp8.bitcast(i32)[:, :],
                    )
                    nc.sync.dma_start(out=xtd[li][t], in_=xstage[:, :])
                else:
                    xt3 = xt.rearrange("p (h c) -> p h c", c=C)
                    m1 = epip.tile([128, 3, C], bf, tag="m1")
                    nc.vector.tensor_tensor(out=m1[:, :, :], in0=xt3[:, 0:3, :],
                                            in1=xt3[:, 3:6, :], op=ALU.add)
                    m2 = epip.tile([128, C], bf, tag="m2")
                    nc.vector.tensor_tensor(out=m2[:, :], in0=m1[:, 0, :], in1=m1[:, 1, :], op=ALU.add)
                    m3 = epip.tile([128, C], f32, tag="m3")
                    nc.vector.tensor_tensor(out=m3[:, :], in0=m2[:, :], in1=m1[:, 2, :], op=ALU.add)
                    ot = epip.tile([128, C], f32, tag="ot")
                    nc.vector.tensor_scalar(out=ot[:, :], in0=m3[:, :], scalar1=1.0 / H,
                                            scalar2=None, op0=ALU.mult)
                    nc.vector.tensor_tensor(out=ot[:r, :], in0=ot[:r, :], in1=lt2[:r, :], op=ALU.add)
                    nc.sync.dma_start(out=out_p[t * 128 : t * 128 + r, :], in_=ot[:r, :])

    nc.finalize()
    return nc


# --------------------------------------------------------------------------
# runner
# --------------------------------------------------------------------------

def _run(inputs, sim=False, trace=False, n_cores=N_CORES, tmpdir=None):
    in_maps, cfg, perm = _host_prep(inputs, n_cores)
    nc = _build(cfg)
    if sim:
        import concourse.bass_interp as bass_interp

        msim = bass_interp.MultiCoreSim(nc, n_cores)
        for c in range(n_cores):
            for k, v in in_maps[c].items():
                msim.cores[c].tensor(k)[:] = v
        msim.simulate(check_with_hw=True)
        outs = [np.array(msim.cores[c].mem_tensor("out")) for c in range(n_cores)]
        exec_ns = None
    else:
        from concourse.bass_utils import run_bass_kernel_spmd

        res = run_bass_kernel_spmd(
            nc, in_maps, list(range(n_cores)), trace=trace, tmpdir=tmpdir
        )
        outs = [res.results[c]["out"] for c in range(n_cores)]
        exec_ns = res.exec_time_ns
    out_new = np.concatenate(outs, 0)
    out = np.empty_like(out_new)
    out[...] = out_new[perm]
    return out.astype(np.float32), exec_ns


def kernel(**inputs) -> np.ndarray:
    out, _ = _run(inputs)
    return out
